# revision 5
# baseline (speedup 1.0000x reference)
"""Trainium2 Bass kernel for nn_LinearEmbedded (moe_routing).

Reference computation:
    w = weight1[region_ix]             # (B, C, D) gather per-region weights
    out = einsum('abc,bcd->abd', x, w) + bias1[region_ix][None]

Sharding: B (128 regions) split across 8 NeuronCores, 16 per core; the
per-region weight/bias gather happens host-side.

v8 scheme (v7 + int8 output path; v6 replaced v5's int8+DVE-convert path):
  - Weights ship as fp8 e3m4 (1 B/elem) with a per-(b,c)-row scale folded
    into x; the PE consumes e3m4 as the MOVING operand directly (validated
    bit-exact on HW, probe_fp8.py), so v5's ~30 us of DVE int8->fp16
    weight converts disappear entirely.  Row scales are picked per-row
    from 6 candidates to minimize l2 error.
  - x ships as int8 (per-(a,b)-row scale t, folded out on the host) and
    is per-b cast int8->fp16 on the otherwise-idle DVE.
  - Output returns as int8: the ACT evac multiplies PSUM by a host-chosen
    per-(a,b) scale k = 127/(4 sigma_est) and converts (probe_i8.py: HW
    is exact round-to-nearest with [-128,127] saturation).  sigma_est =
    ||qx_ab|| * rms_col||qw_b|| / sqrt(C); the host divides k and t back
    out and adds the bias.  All device arithmetic is bit-reproducible
    host-side (int8 x e3m4 products are fp32-exact, |psum| < 2^24), so
    the measured l2 rel err 1.77e-2 (gate 2e-2) is deterministic.
  - The PE runs only the 64 K=128 matmuls (N=512 rows each); HBM traffic
    is 6.3 MB/core ~= 18 us at the measured ~350 B/ns pool rate.

Schedule (per core; DMA-bandwidth-bound, v6/v7 traces):
  - SP issues the 13 loads in stream order [ks x0 w01 w23 x1 w45 w67
    x2(8b) w89 wAB wCD w14 w15]; every load has a dedicated SBUF region
    (no reuse, no flow control).  The last two w chunks are single-b so
    the final matmuls start as early as possible.
  - PE: 13 dummy matmuls bridge the preamble to the first real weights
    (HAM clock-gate warms after ~3.4 us sustained activity; the stream
    then paces the PE at 2.4 GHz -- 215 ns/matmul steady, v6 trace).
  - DVE per-b casts x with lag-by-one consumption on the PE (wait cast
    b+2 + trailing scratch cast) -- the PE read port races a
    just-finished DVE write if it consumes the instant the sem flips
    (observed on HW in v5).
  - ACT evacuates PSUM per-b (scale-by-k ACTIVATE, fp32 -> int8) into a
    dedicated staging region (no slot recycling).
  - Pool stores via SWDGE (no shared-HWDGE contention with the loads).
    The first store also waits on the second-to-last w load so stores
    queue AFTER the loads: the DMA pool round-robins across queues, and
    ungated stores interleave into the load stream and push the last
    weights (and with them the whole PE/evac/store tail) out by several
    us (observed in v7).  Tail stores are single-b to shorten the final
    evac->store chain.
  - SP ends with the completion proof (every DMA retired, every stream
    drained): teardown resets active DMA rings and in-flight descriptors
    hard-fault the device (NRT_EXEC_UNIT_UNRECOVERABLE, observed).
  - The ~7 us after the last store sem is the framework postamble (exit
    barrier + clearing all 256 HW semaphores, ~51 per engine serially);
    it is re-executability teardown and not avoidable from kernel code.
"""

import numpy as np
import ml_dtypes

A, B, C, D = 128, 128, 512, 512
NCORES = 8
BL = B // NCORES   # 16 b per core
KC = C // 128      # 4 contraction chunks
R_PB = 6           # psum banks for real work (+1 warmup dummy bank)
N_WARM = 13        # dummy warmup matmuls (HAM clock ramp + preamble bridge)
F8MAX = 15.5       # e3m4 max finite
OMARGIN = 4.0      # int8 output scale margin (in sigma_est units)
WB = KC * D        # 2048 weight cols per b
XB = KC * A        # 512 x cols per b

# w load chunks: (first b, n bs); last two single so the tail starts early
W_CHUNKS = [(0, 2), (2, 2), (4, 2), (6, 2), (8, 2), (10, 2), (12, 2),
            (14, 1), (15, 1)]
# x load chunks (in units of b)
X_CHUNKS = [(0, 4), (4, 4), (8, 8)]
# out stores: (first b, n bs); fine-grained at the tail
O_CHUNKS = [(0, 4), (4, 4), (8, 2), (10, 2), (12, 1), (13, 1), (14, 1),
            (15, 1)]
# SP issue order: x chunks slotted so they arrive just ahead of need
LOAD_ORDER = ["x0", "w0", "w1", "x1", "w2", "w3", "x2", "w4", "w5", "w6",
              "w7", "w8"]

_prog = None


def _build_program():
    global _prog
    if _prog is not None:
        return _prog

    import concourse.bass as bass
    import concourse.mybir as mybir
    from contextlib import ExitStack

    F32 = mybir.dt.float32
    F16 = mybir.dt.float16
    F8 = mybir.dt.float8e3
    I8 = mybir.dt.int8
    nc = bass.Bass("TRN2", target_bir_lowering=False, debug=False)

    w8 = nc.dram_tensor("w8", [128, BL * WB], F8, kind="ExternalInput")
    xq = nc.dram_tensor("xq", [128, BL * XB], I8, kind="ExternalInput")
    ks = nc.dram_tensor("ks", [128, BL], F32, kind="ExternalInput")
    out = nc.dram_tensor("out", [A, BL * D], I8, kind="ExternalOutput")

    ctx = ExitStack()
    with ctx:
        ws = ctx.enter_context(nc.sbuf_tensor("ws", [128, BL * WB], F8))
        xs8 = ctx.enter_context(nc.sbuf_tensor("xs8", [128, BL * XB], I8))
        xs = ctx.enter_context(nc.sbuf_tensor("xs", [128, BL * XB], F16))
        ks_s = ctx.enter_context(nc.sbuf_tensor("ks_s", [128, BL], F32))
        ots = ctx.enter_context(nc.sbuf_tensor("ots", [128, BL * D], I8))
        wrm = ctx.enter_context(nc.sbuf_tensor("wrm", [128, 128 + D], F16))
        psum = ctx.enter_context(nc.psum_tensor("psum", [A, R_PB * D], F32))
        psum_d = ctx.enter_context(nc.psum_tensor("psum_d", [A, D], F32))

        s_w = [ctx.enter_context(nc.semaphore(f"s_w{p}"))
               for p in range(len(W_CHUNKS))]
        s_x = [ctx.enter_context(nc.semaphore(f"s_x{q}"))
               for q in range(len(X_CHUNKS))]
        s_ks = ctx.enter_context(nc.semaphore("s_ks"))
        s_xc = ctx.enter_context(nc.semaphore("s_xc"))  # +1 per DVE cast
        s_pe = ctx.enter_context(nc.semaphore("s_pe"))  # +1 per finished b
        s_cp = ctx.enter_context(nc.semaphore("s_cp"))  # +1 per PSUM evac
        s_st = ctx.enter_context(nc.semaphore("s_st"))  # +16 per store

        sync, scalar, tensor, vector, pool = (
            nc.sync, nc.scalar, nc.tensor, nc.vector, nc.gpsimd,
        )

        # which w chunk feeds each b
        w_of_b = {}
        for ci, (b0, nb) in enumerate(W_CHUNKS):
            for b in range(b0, b0 + nb):
                w_of_b[b] = ci

        # --- SP: all loads, no flow control (distinct SBUF regions) ---
        sync.dma_start(ks_s[:], ks[:]).then_inc(s_ks, 16)
        for item in LOAD_ORDER:
            i = int(item[1:])
            if item[0] == "x":
                b0, nb = X_CHUNKS[i]
                sync.dma_start(
                    xs8[:, b0 * XB : (b0 + nb) * XB],
                    xq[:, b0 * XB : (b0 + nb) * XB],
                ).then_inc(s_x[i], 16)
            else:
                b0, nb = W_CHUNKS[i]
                sync.dma_start(
                    ws[:, b0 * WB : (b0 + nb) * WB],
                    w8[:, b0 * WB : (b0 + nb) * WB],
                ).then_inc(s_w[i], 16)

        # tail: completion proof (see module docstring)
        sync.wait_ge(s_pe, BL)
        sync.wait_ge(s_cp, BL)
        sync.wait_ge(s_xc, BL + 1)
        sync.wait_ge(s_ks, 16)
        for p in range(len(W_CHUNKS)):
            sync.wait_ge(s_w[p], 16)
        for q in range(len(X_CHUNKS)):
            sync.wait_ge(s_x[q], 16)
        sync.wait_ge(s_st, 16 * len(O_CHUNKS))

        # --- DVE: 16 per-b casts int8 -> fp16 + trailing scratch cast ---
        for b in range(BL):
            xcl = [i for i, (b0, nb) in enumerate(X_CHUNKS)
                   if b0 <= b < b0 + nb][0]
            vector.wait_ge(s_x[xcl], 16)
            nc.vector.tensor_copy(
                xs[:, b * XB : (b + 1) * XB],
                xs8[:, b * XB : (b + 1) * XB],
            ).then_inc(s_xc, 1)
        # scratch cast: gives b15's consumer the same lag-by-one slack
        # (xs8[:, :64] is dead -- cast 0 already consumed it)
        nc.vector.tensor_copy(xs8[:, 0:64], xs8[:, 64:128]).then_inc(s_xc, 1)

        # --- PE: warmup dummies + 4 K=128 matmuls per b ---
        for _ in range(N_WARM):
            nc.tensor.matmul(
                psum_d[:], wrm[:, 0:128], wrm[:, 128 : 128 + D],
                start=True, stop=True,
            )
        waited_w = set()
        for b in range(BL):
            if b >= R_PB:
                tensor.wait_ge(s_cp, b - R_PB + 1)
            wc = w_of_b[b]
            if wc not in waited_w:
                waited_w.add(wc)
                tensor.wait_ge(s_w[wc], 16)
            tensor.wait_ge(s_xc, b + 2)  # lag-by-one on DVE casts
            for k in range(KC):
                mm = nc.tensor.matmul(
                    psum[:, (b % R_PB) * D : (b % R_PB) * D + D],
                    xs[:, b * XB + k * A : b * XB + (k + 1) * A],
                    ws[:, b * WB + k * D : b * WB + (k + 1) * D],
                    start=(k == 0),
                    stop=(k == KC - 1),
                )
            mm.then_inc(s_pe, 1)

        # --- ACT: per-b PSUM evac, scale-by-k, fp32 -> int8 ---
        scalar.wait_ge(s_ks, 16)
        for b in range(BL):
            scalar.wait_ge(s_pe, b + 1)
            nc.scalar.activation(
                ots[:, b * D : (b + 1) * D],
                psum[:, (b % R_PB) * D : (b % R_PB) * D + D],
                mybir.ActivationFunctionType.Copy,
                scale=ks_s[:, b : b + 1],
            ).then_inc(s_cp, 1)

        # --- Pool: stores via SWDGE, gated behind the load stream ---
        pool.wait_ge(s_w[len(W_CHUNKS) - 2], 16)
        for b0, nb in O_CHUNKS:
            pool.wait_ge(s_cp, b0 + nb)
            pool.dma_start(
                out[:, b0 * D : (b0 + nb) * D], ots[:, b0 * D : (b0 + nb) * D]
            ).then_inc(s_st, 16)

    _prog = nc
    return nc


_RATIOS = np.array([1.0, 0.97, 0.94, 0.91, 0.88, 0.85], dtype=np.float32)


def _quant_w_e3m4(wg):
    """Per-(b,c)-row e3m4 quantization with l2-optimal scale from 6
    candidates.  Returns (qw float8_e3m4 (BL,C,D), s (BL,C) fp32)."""
    f8 = ml_dtypes.float8_e3m4
    wmax = np.maximum(np.abs(wg).max(axis=2), 1e-30)  # (BL,C)
    best_err = None
    best_s = None
    best_q = None
    for r in _RATIOS:
        s = (wmax / (F8MAX * r)).astype(np.float32)
        q = np.clip(wg / s[:, :, None], -F8MAX, F8MAX).astype(f8)
        e = ((q.astype(np.float32) * s[:, :, None] - wg) ** 2).sum(axis=2)
        if best_err is None:
            best_err, best_s, best_q = e, s, q
        else:
            m = e < best_err
            best_err = np.where(m, e, best_err)
            best_s = np.where(m, s, best_s)
            best_q[m] = q[m]
    return best_q, best_s


def _shard_inputs(x, region_ix, weight1, bias1):
    in_maps = []
    post = []  # (t/k factor, bias) per core for host-side un-scaling
    for c in range(NCORES):
        bs = slice(c * BL, (c + 1) * BL)
        rloc = region_ix[bs]
        wg = weight1[rloc]                        # (BL, C, D) f32
        qw, s = _quant_w_e3m4(wg)
        qwf = qw.astype(np.float32)
        # device layout: per b [128 part (c within chunk), KC*D], b-major
        wdev = np.ascontiguousarray(
            qw.reshape(BL, KC, 128, D).transpose(2, 0, 1, 3)
        ).reshape(128, BL * WB)
        # x: fold s, int8 per-(a,b)-row
        xp = x[:, bs, :] * s[None, :, :]          # (A, BL, C)
        t = np.maximum(np.abs(xp).max(axis=2), 1e-30) / 127.0   # (A, BL)
        qx = np.clip(np.rint(xp / t[:, :, None]), -127, 127).astype(np.int8)
        # device layout: per b [128 part (c within chunk), KC*A], b-major
        xt = np.ascontiguousarray(
            qx.transpose(1, 2, 0).reshape(BL, KC, 128, A).transpose(2, 0, 1, 3)
        ).reshape(128, BL * XB)
        # int8 output scale k = 127/(OMARGIN * sigma_est) per (a, b)
        wcol2 = (qwf.astype(np.float64) ** 2).sum(axis=1).mean(axis=1)  # (BL,)
        sig = (
            np.linalg.norm(qx.astype(np.float32), axis=2)
            * np.sqrt(wcol2)[None].astype(np.float32)
            / np.sqrt(C)
        )                                         # (A, BL)
        sig = np.maximum(sig, 1e-20)
        kv = (127.0 / (OMARGIN * sig)).astype(np.float32)
        in_maps.append({"w8": wdev, "xq": xt, "ks": kv})
        post.append((t / kv, bias1[rloc]))
    return in_maps, post


def kernel(x, region_ix, weight1, bias1):
    from concourse.bass_utils import run_bass_kernel_spmd

    x = np.asarray(x, dtype=np.float32)
    region_ix = np.asarray(region_ix).astype(np.int64)
    weight1 = np.asarray(weight1, dtype=np.float32)
    bias1 = np.asarray(bias1, dtype=np.float32)

    nc = _build_program()
    in_maps, post = _shard_inputs(x, region_ix, weight1, bias1)
    res = run_bass_kernel_spmd(nc, in_maps, core_ids=list(range(NCORES)))

    outv = np.empty((A, B, D), dtype=np.float32)
    for c in range(NCORES):
        f, bg = post[c]
        q = res.results[c]["out"].reshape(A, BL, D).astype(np.float32)
        outv[:, c * BL : (c + 1) * BL, :] = q * f[:, :, None] + bg[None]
    return outv


# revision 11
# speedup vs baseline: 1.0325x; 1.0325x over previous
"""Trainium2 Bass kernel for nn_LinearEmbedded (moe_routing).

Reference computation:
    w = weight1[region_ix]             # (B, C, D) gather per-region weights
    out = einsum('abc,bcd->abd', x, w) + bias1[region_ix][None]

Sharding: B (128 regions) split across 8 NeuronCores, 16 per core; the
per-region weight/bias gather happens host-side.

v8 scheme (v7 + int8 output path; v6 replaced v5's int8+DVE-convert path):
  - Weights ship as fp8 e3m4 (1 B/elem) with a per-(b,c)-row scale folded
    into x; the PE consumes e3m4 as the MOVING operand directly (validated
    bit-exact on HW, probe_fp8.py), so v5's ~30 us of DVE int8->fp16
    weight converts disappear entirely.  Row scales are picked per-row
    from 6 candidates to minimize l2 error.
  - x ships as int8 (per-(a,b)-row scale t, folded out on the host) and
    is per-b cast int8->fp16 on the otherwise-idle DVE.
  - Output returns as int8: the ACT evac multiplies PSUM by a host-chosen
    per-(a,b) scale k = 127/(4 sigma_est) and converts (probe_i8.py: HW
    is exact round-to-nearest with [-128,127] saturation).  sigma_est =
    ||qx_ab|| * rms_col||qw_b|| / sqrt(C); the host divides k and t back
    out and adds the bias.  All device arithmetic is bit-reproducible
    host-side (int8 x e3m4 products are fp32-exact, |psum| < 2^24), so
    the measured l2 rel err 1.77e-2 (gate 2e-2) is deterministic.
  - The PE runs only the 64 K=128 matmuls (N=512 rows each); HBM traffic
    is 6.3 MB/core ~= 18 us at the measured ~350 B/ns pool rate.

Schedule (per core; DMA-bandwidth-bound, v6/v7 traces):
  - SP issues the 13 loads in stream order [ks x0 w01 w23 x1 w45 w67
    x2(8b) w89 wAB wCD w14 w15]; every load has a dedicated SBUF region
    (no reuse, no flow control).  The last two w chunks are single-b so
    the final matmuls start as early as possible.
  - PE: 13 dummy matmuls bridge the preamble to the first real weights
    (HAM clock-gate warms after ~3.4 us sustained activity; the stream
    then paces the PE at 2.4 GHz -- 215 ns/matmul steady, v6 trace).
  - DVE per-b casts x with lag-by-one consumption on the PE (wait cast
    b+2 + trailing scratch cast) -- the PE read port races a
    just-finished DVE write if it consumes the instant the sem flips
    (observed on HW in v5).
  - ACT evacuates PSUM per-b (scale-by-k ACTIVATE, fp32 -> int8) into a
    dedicated staging region (no slot recycling).
  - Pool stores via SWDGE (no shared-HWDGE contention with the loads).
    The first store also waits on the second-to-last w load so stores
    queue AFTER the loads: the DMA pool round-robins across queues, and
    ungated stores interleave into the load stream and push the last
    weights (and with them the whole PE/evac/store tail) out by several
    us (observed in v7).  Tail stores are single-b to shorten the final
    evac->store chain.
  - SP ends with the completion proof (every DMA retired, every stream
    drained): teardown resets active DMA rings and in-flight descriptors
    hard-fault the device (NRT_EXEC_UNIT_UNRECOVERABLE, observed).
  - The ~7 us after the last store sem is the framework postamble (exit
    barrier + clearing all 256 HW semaphores, ~51 per engine serially);
    it is re-executability teardown and not avoidable from kernel code.
"""

import numpy as np
import ml_dtypes

A, B, C, D = 128, 128, 512, 512
NCORES = 8
BL = B // NCORES   # 16 b per core
KC = C // 128      # 4 contraction chunks
R_PB = 6           # psum banks for real work (+1 warmup dummy bank)
N_WARM = 13        # dummy warmup matmuls (HAM clock ramp + preamble bridge)
F8MAX = 15.5       # e3m4 max finite
OMARGIN = 4.0      # int8 output scale margin (in sigma_est units)
WB = KC * D        # 2048 weight cols per b
XB = KC * A        # 512 x cols per b

# w load chunks: (first b, n bs); last two single so the tail starts early
W_CHUNKS = [(0, 2), (2, 2), (4, 2), (6, 2), (8, 2), (10, 2), (12, 2),
            (14, 1), (15, 1)]
# x load chunks (in units of b)
X_CHUNKS = [(0, 4), (4, 4), (8, 8)]
# out stores: (first b, n bs); batches on Pool, tail singles on SP
O_POOL = [(0, 8), (8, 2), (10, 2)]
O_SP = [(12, 1), (13, 1), (14, 1), (15, 1)]
N_EVAC_ACT = 10    # ACT evacuates b0..b9; DVE (free after casts) b10..b15
# SP issue order: x chunks slotted so they arrive just ahead of need
LOAD_ORDER = ["x0", "w0", "w1", "x1", "w2", "w3", "x2", "w4", "w5", "w6",
              "w7", "w8"]

_prog = None


def _build_program():
    global _prog
    if _prog is not None:
        return _prog

    import concourse.bass as bass
    import concourse.mybir as mybir
    from contextlib import ExitStack

    F32 = mybir.dt.float32
    F16 = mybir.dt.float16
    F8 = mybir.dt.float8e3
    I8 = mybir.dt.int8
    nc = bass.Bass("TRN2", target_bir_lowering=False, debug=False)

    w8 = nc.dram_tensor("w8", [128, BL * WB], F8, kind="ExternalInput")
    xq = nc.dram_tensor("xq", [128, BL * XB], I8, kind="ExternalInput")
    ks = nc.dram_tensor("ks", [128, BL], F32, kind="ExternalInput")
    out = nc.dram_tensor("out", [A, BL * D], I8, kind="ExternalOutput")

    ctx = ExitStack()
    with ctx:
        ws = ctx.enter_context(nc.sbuf_tensor("ws", [128, BL * WB], F8))
        xs8 = ctx.enter_context(nc.sbuf_tensor("xs8", [128, BL * XB], I8))
        xs = ctx.enter_context(nc.sbuf_tensor("xs", [128, BL * XB], F16))
        ks_s = ctx.enter_context(nc.sbuf_tensor("ks_s", [128, BL], F32))
        ots = ctx.enter_context(nc.sbuf_tensor("ots", [128, BL * D], I8))
        wrm = ctx.enter_context(nc.sbuf_tensor("wrm", [128, 128 + D], F16))
        psum = ctx.enter_context(nc.psum_tensor("psum", [A, R_PB * D], F32))
        psum_d = ctx.enter_context(nc.psum_tensor("psum_d", [A, D], F32))

        s_w = [ctx.enter_context(nc.semaphore(f"s_w{p}"))
               for p in range(len(W_CHUNKS))]
        s_x = [ctx.enter_context(nc.semaphore(f"s_x{q}"))
               for q in range(len(X_CHUNKS))]
        s_ks = ctx.enter_context(nc.semaphore("s_ks"))
        s_xc = ctx.enter_context(nc.semaphore("s_xc"))  # +1 per DVE cast
        s_pe = ctx.enter_context(nc.semaphore("s_pe"))  # +1 per finished b
        s_cp = ctx.enter_context(nc.semaphore("s_cp"))  # +1 per ACT evac
        s_cpv = ctx.enter_context(nc.semaphore("s_cpv"))  # +1 per DVE evac
        s_st = ctx.enter_context(nc.semaphore("s_st"))  # +16 per store

        sync, scalar, tensor, vector, pool = (
            nc.sync, nc.scalar, nc.tensor, nc.vector, nc.gpsimd,
        )

        # which w chunk feeds each b
        w_of_b = {}
        for ci, (b0, nb) in enumerate(W_CHUNKS):
            for b in range(b0, b0 + nb):
                w_of_b[b] = ci

        # --- SP: all loads, no flow control (distinct SBUF regions) ---
        for item in LOAD_ORDER:
            i = int(item[1:])
            if item[0] == "x":
                b0, nb = X_CHUNKS[i]
                sync.dma_start(
                    xs8[:, b0 * XB : (b0 + nb) * XB],
                    xq[:, b0 * XB : (b0 + nb) * XB],
                ).then_inc(s_x[i], 16)
            else:
                b0, nb = W_CHUNKS[i]
                sync.dma_start(
                    ws[:, b0 * WB : (b0 + nb) * WB],
                    w8[:, b0 * WB : (b0 + nb) * WB],
                ).then_inc(s_w[i], 16)

        # tail singles: SP's HWDGE queue is idle after the loads, and
        # issuing them here runs the SWDGE-gen-paced Pool queue and this
        # one in parallel through the store tail
        for b0, nb in O_SP:
            sync.wait_ge(s_cpv, b0 + nb - N_EVAC_ACT)
            sync.dma_start(
                out[:, b0 * D : (b0 + nb) * D], ots[:, b0 * D : (b0 + nb) * D]
            ).then_inc(s_st, 16)

        # tail: completion proof (see module docstring)
        sync.wait_ge(s_pe, BL)
        sync.wait_ge(s_cp, N_EVAC_ACT)
        sync.wait_ge(s_cpv, BL - N_EVAC_ACT)
        sync.wait_ge(s_xc, BL + 1)
        sync.wait_ge(s_ks, 16)
        for p in range(len(W_CHUNKS)):
            sync.wait_ge(s_w[p], 16)
        for q in range(len(X_CHUNKS)):
            sync.wait_ge(s_x[q], 16)
        sync.wait_ge(s_st, 16 * (len(O_POOL) + len(O_SP)))

        # --- DVE: 16 per-b casts int8 -> fp16 + trailing scratch cast ---
        for b in range(BL):
            xcl = [i for i, (b0, nb) in enumerate(X_CHUNKS)
                   if b0 <= b < b0 + nb][0]
            vector.wait_ge(s_x[xcl], 16)
            nc.vector.tensor_copy(
                xs[:, b * XB : (b + 1) * XB],
                xs8[:, b * XB : (b + 1) * XB],
            ).then_inc(s_xc, 1)
        # scratch cast: gives b15's consumer the same lag-by-one slack
        # (xs8[:, :64] is dead -- cast 0 already consumed it)
        nc.vector.tensor_copy(xs8[:, 0:64], xs8[:, 64:128]).then_inc(s_xc, 1)
        # DVE evacuates the tail b's (ACT's serial evac chain would
        # otherwise pace the PE tail through psum-bank recycling)
        vector.wait_ge(s_ks, 16)
        for b in range(N_EVAC_ACT, BL):
            vector.wait_ge(s_pe, b + 1)
            nc.vector.tensor_scalar(
                ots[:, b * D : (b + 1) * D],
                psum[:, (b % R_PB) * D : (b % R_PB) * D + D],
                ks_s[:, b : b + 1],
                None,
                mybir.AluOpType.mult,
            ).then_inc(s_cpv, 1)

        # --- PE: warmup dummies + 4 K=128 matmuls per b ---
        for _ in range(N_WARM):
            nc.tensor.matmul(
                psum_d[:], wrm[:, 0:128], wrm[:, 128 : 128 + D],
                start=True, stop=True,
            )
        waited_w = set()
        for b in range(BL):
            if b >= R_PB:
                tensor.wait_ge(s_cp, b - R_PB + 1)
            wc = w_of_b[b]
            if wc not in waited_w:
                waited_w.add(wc)
                tensor.wait_ge(s_w[wc], 16)
            tensor.wait_ge(s_xc, b + 2)  # lag-by-one on DVE casts
            for k in range(KC):
                mm = nc.tensor.matmul(
                    psum[:, (b % R_PB) * D : (b % R_PB) * D + D],
                    xs[:, b * XB + k * A : b * XB + (k + 1) * A],
                    ws[:, b * WB + k * D : b * WB + (k + 1) * D],
                    start=(k == 0),
                    stop=(k == KC - 1),
                )
            mm.then_inc(s_pe, 1)

        # --- ACT: ks load, act-table pre-warm, per-b PSUM evac b0..b9 ---
        scalar.dma_start(ks_s[:], ks[:]).then_inc(s_ks, 16)
        # dummy ACTIVATE so the 1.3 us activation-table load happens at
        # t~0, not in front of the first real evac (observed in v8 trace)
        nc.scalar.activation(
            wrm[:, 0:1], wrm[:, 1:2], mybir.ActivationFunctionType.Copy
        )
        scalar.wait_ge(s_ks, 16)
        for b in range(N_EVAC_ACT):
            scalar.wait_ge(s_pe, b + 1)
            nc.scalar.activation(
                ots[:, b * D : (b + 1) * D],
                psum[:, (b % R_PB) * D : (b % R_PB) * D + D],
                mybir.ActivationFunctionType.Copy,
                scale=ks_s[:, b : b + 1],
            ).then_inc(s_cp, 1)

        # --- Pool: batch stores via SWDGE, gated behind the load stream
        # (the DMA pool round-robins across queues; ungated stores
        # interleave into the loads and push the whole tail out) ---
        pool.wait_ge(s_w[len(W_CHUNKS) - 2], 16)
        for b0, nb in O_POOL:
            last = b0 + nb - 1
            if last < N_EVAC_ACT:
                pool.wait_ge(s_cp, last + 1)
            else:
                pool.wait_ge(s_cpv, last + 1 - N_EVAC_ACT)
            pool.dma_start(
                out[:, b0 * D : (b0 + nb) * D], ots[:, b0 * D : (b0 + nb) * D]
            ).then_inc(s_st, 16)

    _prog = nc
    return nc


_RATIOS = np.array([1.0, 0.97, 0.94, 0.91, 0.88, 0.85], dtype=np.float32)


def _quant_w_e3m4(wg):
    """Per-(b,c)-row e3m4 quantization with l2-optimal scale from 6
    candidates.  Returns (qw float8_e3m4 (BL,C,D), s (BL,C) fp32)."""
    f8 = ml_dtypes.float8_e3m4
    wmax = np.maximum(np.abs(wg).max(axis=2), 1e-30)  # (BL,C)
    best_err = None
    best_s = None
    best_q = None
    for r in _RATIOS:
        s = (wmax / (F8MAX * r)).astype(np.float32)
        q = np.clip(wg / s[:, :, None], -F8MAX, F8MAX).astype(f8)
        e = ((q.astype(np.float32) * s[:, :, None] - wg) ** 2).sum(axis=2)
        if best_err is None:
            best_err, best_s, best_q = e, s, q
        else:
            m = e < best_err
            best_err = np.where(m, e, best_err)
            best_s = np.where(m, s, best_s)
            best_q[m] = q[m]
    return best_q, best_s


def _shard_inputs(x, region_ix, weight1, bias1):
    in_maps = []
    post = []  # (t/k factor, bias) per core for host-side un-scaling
    for c in range(NCORES):
        bs = slice(c * BL, (c + 1) * BL)
        rloc = region_ix[bs]
        wg = weight1[rloc]                        # (BL, C, D) f32
        qw, s = _quant_w_e3m4(wg)
        qwf = qw.astype(np.float32)
        # device layout: per b [128 part (c within chunk), KC*D], b-major
        wdev = np.ascontiguousarray(
            qw.reshape(BL, KC, 128, D).transpose(2, 0, 1, 3)
        ).reshape(128, BL * WB)
        # x: fold s, int8 per-(a,b)-row
        xp = x[:, bs, :] * s[None, :, :]          # (A, BL, C)
        t = np.maximum(np.abs(xp).max(axis=2), 1e-30) / 127.0   # (A, BL)
        qx = np.clip(np.rint(xp / t[:, :, None]), -127, 127).astype(np.int8)
        # device layout: per b [128 part (c within chunk), KC*A], b-major
        xt = np.ascontiguousarray(
            qx.transpose(1, 2, 0).reshape(BL, KC, 128, A).transpose(2, 0, 1, 3)
        ).reshape(128, BL * XB)
        # int8 output scale k = 127/(OMARGIN * sigma_est) per (a, b)
        wcol2 = (qwf.astype(np.float64) ** 2).sum(axis=1).mean(axis=1)  # (BL,)
        sig = (
            np.linalg.norm(qx.astype(np.float32), axis=2)
            * np.sqrt(wcol2)[None].astype(np.float32)
            / np.sqrt(C)
        )                                         # (A, BL)
        sig = np.maximum(sig, 1e-20)
        kv = (127.0 / (OMARGIN * sig)).astype(np.float32)
        in_maps.append({"w8": wdev, "xq": xt, "ks": kv})
        post.append((t / kv, bias1[rloc]))
    return in_maps, post


def kernel(x, region_ix, weight1, bias1):
    from concourse.bass_utils import run_bass_kernel_spmd

    x = np.asarray(x, dtype=np.float32)
    region_ix = np.asarray(region_ix).astype(np.int64)
    weight1 = np.asarray(weight1, dtype=np.float32)
    bias1 = np.asarray(bias1, dtype=np.float32)

    nc = _build_program()
    in_maps, post = _shard_inputs(x, region_ix, weight1, bias1)
    res = run_bass_kernel_spmd(nc, in_maps, core_ids=list(range(NCORES)))

    outv = np.empty((A, B, D), dtype=np.float32)
    for c in range(NCORES):
        f, bg = post[c]
        q = res.results[c]["out"].reshape(A, BL, D).astype(np.float32)
        outv[:, c * BL : (c + 1) * BL, :] = q * f[:, :, None] + bg[None]
    return outv
